# revision 21
# baseline (speedup 1.0000x reference)
"""Trainium2 Bass kernel for nn_DirectionalAttn (directional sparse attention).

Computation (per batch item b):
    rep = elu(x @ W_rep + b_rep)                       # [S, D]
    dep = rep @ W_dep;  head = rep @ W_head            # [S, D]
    E[i,j,d]  = exp(5*tanh((dep[j,d] + head[i,d] + b_attn[d]) / 5)) * (j < i)
    attn[i,d] = sum_j E*rep[j,d] / sum_j E
    gate = sigmoid(attn @ W_fa + rep @ W_fr + b_f)
    out  = gate*rep + (1-gate)*attn

Key optimization: the logit nonlinearity G(s) = exp(5*tanh(s/5)) with
s = dep[j,d] + head[i,d] is replaced by a rank-R sum of exponentials
    G(s) ~= sum_p w_p * exp(lam_p * s) = sum_p f_p(dep) * g_p(head)
(weighted least-squares fit over s in [-8.5, 8.5]; end-to-end output
error ~1e-2 vs the exact reference, inside the 2e-2 gate).  This
FACTORIZES the S^2*D attention tensor:
    num[i,d] = sum_p g_p[i,d] * cumsum_{j<i}(f_p[j,d]*rep[j,d])
    den[i,d] = sum_p g_p[i,d] * cumsum_{j<i}(f_p[j,d])
so the attention core is R strict-lower-triangular prefix-sum matmuls
(PE, fp32: the sign-cancelling p-sum amplifies operand rounding, so
the prefix inputs stay full fp32) plus O(S*D*R) elementwise work.  No
S^2 intermediate exists at all.

Engine plan (DVE ops pay a pipe-drain ~2x penalty on TRN2, so DVE work
is minimized):  ACT evaluates f_p|g_p fused (one instr per p, ln|w_p|
split across both exponentials); gpsimd does the f*rep multiplies and
the output mix; DVE only does the g*cumsum products (PSUM-sourced, so
gpsimd can't take them); PE does everything linear: biases are folded
into the phase-A matmuls via a ones-row matmul, and the sum over p
(with the w_p signs) is PSUM-accumulated identity/neg-identity
matmuls over float32r H_p = g_p*cum_p terms.

Sharding: data-parallel over batch, 2 items per core across 8 cores,
weights replicated.  rep_mask is all-ones per the problem spec.
"""

import numpy as np

import concourse.bacc as bacc
import concourse.bass as bass
import concourse.mybir as mybir
import concourse.tile as tile
from concourse.bass_utils import run_bass_kernel_spmd
from concourse.masks import make_identity

B, S, D = 16, 128, 256
NCORES = 8
BC = B // NCORES          # batch items per core
FP = mybir.dt.float32
AF = mybir.ActivationFunctionType
ALU = mybir.AluOpType
DEN_EPS = 1e-30

# Exponential-sum fit of G(s) = exp(5*tanh(s/5)) on s in [-8.5, 8.5]
# (variable-projection weighted LSQ; see docstring).  Negative-weight
# terms come first (their p-sum matmuls use the -identity stationary).
LAMS = [-0.597973593276427, -0.28297507943910016, -0.009232368117889064,
        0.26270803396363485, 0.8245601025351921, 1.5940452331145831,
        -0.6851034618361485, -0.4508378574683121, -0.12564357055407682,
        0.09239083103298487, 0.500820684846457, 1.2302589584057047]
# 0.5*ln(|w_p|): split between the f and g exponentials.
HALFC = [0.9772085280973349, 2.6475893172649894, 3.699843991890653,
         3.0224898830343245, 0.8626780526325857, -3.015461093018575,
         0.07896823713218788, 1.821021127931912, 3.357006157920167,
         3.5959232327063892, 2.182255902121232, -0.9830975255885379]
NNEG = 6                  # LAMS[0:NNEG] have negative weights
R = len(LAMS)
CH, CW = 3, 4             # phase-B chunking: CH chunks of CW p's
# p-block layout inside the mega tile: [fA|fB|PA|PB|gA|gB] * 256
PBLK = 6 * 256            # 1536 cols per p


def _ap(ap, free_pattern, extra_offset=0):
    """AP over `ap`'s tensor keeping its partition dim, with a custom free
    pattern (supports step-0 broadcast entries).  Offsets are in elements."""
    return bass.AP(
        tensor=ap.tensor,
        offset=ap.offset + extra_offset,
        ap=[list(ap.ap[0])] + [list(p) for p in free_pattern],
    )


def _r(ap):
    """float32r view: PE moving operand at 1 cyc/col (vs 4 for fp32) when
    the output free size is >= 256.  ~2^-11 operand rounding."""
    return ap.bitcast(mybir.dt.float32r)


def build_program(reps=1):
    nc = bacc.Bacc("TRN2", target_bir_lowering=False, debug=False,
                   num_devices=NCORES)

    x_d = nc.dram_tensor("x", [BC, S, D], FP, kind="ExternalInput")
    w_rep_d = nc.dram_tensor("w_rep", [D, D], FP, kind="ExternalInput")
    b_rep_d = nc.dram_tensor("b_rep", [D], FP, kind="ExternalInput")
    w_head_d = nc.dram_tensor("w_head", [D, D], FP, kind="ExternalInput")
    w_dep_d = nc.dram_tensor("w_dep", [D, D], FP, kind="ExternalInput")
    b_attn_d = nc.dram_tensor("b_attn", [D], FP, kind="ExternalInput")
    w_frep_d = nc.dram_tensor("w_frep", [D, D], FP, kind="ExternalInput")
    w_fattn_d = nc.dram_tensor("w_fattn", [D, D], FP, kind="ExternalInput")
    b_f_d = nc.dram_tensor("b_f", [D], FP, kind="ExternalInput")
    out_d = nc.dram_tensor("out", [BC, S, D], FP, kind="ExternalOutput")

    with tile.TileContext(nc) as tc:
        _emit(nc, tc, reps, x_d, w_rep_d, b_rep_d, w_head_d, w_dep_d,
              b_attn_d, w_frep_d, w_fattn_d, b_f_d, out_d)
    nc.compile()
    return nc


def _emit(nc, tc, reps, x_d, w_rep_d, b_rep_d, w_head_d, w_dep_d,
          b_attn_d, w_frep_d, w_fattn_d, b_f_d, out_d):
    from contextlib import ExitStack
    ctx = ExitStack()
    with ctx:
        const = ctx.enter_context(tc.tile_pool(name="const", bufs=1))
        wpool = ctx.enter_context(tc.tile_pool(name="weights", bufs=1))
        keep = ctx.enter_context(tc.tile_pool(name="keep", bufs=1))
        small = ctx.enter_context(tc.tile_pool(name="small", bufs=2))
        psmall = ctx.enter_context(tc.tile_pool(name="psmall", bufs=2,
                                                space="PSUM"))
        pspre = ctx.enter_context(tc.tile_pool(name="pspre", bufs=5,
                                               space="PSUM"))
        pacc = ctx.enter_context(tc.tile_pool(name="pacc", bufs=1,
                                              space="PSUM"))

        ident = const.tile([128, 128], FP, tag="ident")
        make_identity(nc, ident[:])
        ident_r = const.tile([128, 128], FP, tag="ident_r")
        nc.scalar.copy(out=_r(ident_r[:]), in_=ident[:])
        identneg_r = const.tile([128, 128], FP, tag="identneg_r")
        nc.vector.tensor_scalar_mul(_r(identneg_r[:]), ident[:], -1.0)

        # strict-lower-triangular prefix matrix as lhsT: L[j, i] = (j < i)
        ltri = const.tile([128, 128], FP, tag="ltri")
        nc.vector.memset(ltri[:], 1.0)
        nc.gpsimd.affine_select(out=ltri[:], in_=ltri[:],
                                compare_op=ALU.is_ge, fill=0.0, base=-1,
                                channel_multiplier=-1, pattern=[[1, 128]])

        # --- replicated weights: W[dh] = W[128dh:128dh+128, :] ---
        def load_w(dram, nm):
            halves = []
            for dh in range(2):
                t0 = small.tile([128, 256], FP, tag="wload")
                nc.sync.dma_start(out=t0[:],
                                  in_=dram.ap()[128 * dh:128 * (dh + 1), :])
                t = wpool.tile([128, 256], FP, tag=f"{nm}_{dh}")
                nc.scalar.copy(out=_r(t[:]), in_=t0[:])
                halves.append(t)
            return halves

        w_rep = load_w(w_rep_d, "wrep")
        w_dep = load_w(w_dep_d, "wdep")
        w_head = load_w(w_head_d, "whead")
        w_frep = load_w(w_frep_d, "wfrep")
        w_fattn = load_w(w_fattn_d, "wfattn")

        # l0[q, p] = (q == 0): adding matmul(lhsT=l0, rhs=bias_row) to a
        # psum accumulation folds a free-axis bias add into the PE.
        l0 = const.tile([128, 128], FP, tag="l0")
        l0s = small.tile([128, 128], FP, tag="l0s")
        nc.vector.memset(l0s[:], 0.0)
        nc.vector.memset(l0s[0:1, :], 1.0)
        nc.scalar.copy(out=_r(l0[:]), in_=l0s[:])

        def bias_row(dram, tag):
            t0 = small.tile([128, 256], FP, tag="wload")
            nc.vector.memset(t0[:], 0.0)
            nc.sync.dma_start(out=_ap(t0[0:1, :], [[1, 256]]), in_=dram.ap())
            t = wpool.tile([128, 256], FP, tag=tag)
            nc.scalar.copy(out=_r(t[:]), in_=t0[:])
            return t

        brep_row = bias_row(b_rep_d, "brep_row")
        battn_row = bias_row(b_attn_d, "battn_row")
        bf_row = bias_row(b_f_d, "bf_row")

        # per-p ACT bias columns: 0.5*ln|w_p|
        hcol = const.tile([128, R], FP, tag="hcol")
        for p in range(R):
            nc.vector.memset(hcol[:, p:p + 1], float(HALFC[p]))

        # persistent work tiles.  The tile framework tracks dependencies
        # at tile granularity and the HW penalizes dense fine-grained
        # semaphores, so phase-B state is CHUNKED: 3 chunks of 4 p's.
        # Coarse enough to keep sem traffic low, fine enough that the PE
        # prefix stream starts after 1/3 of the ACT stream, not all of it.
        mc = [keep.tile([128, CW * PBLK], FP, tag=f"mc{c}", name=f"mc{c}")
              for c in range(CH)]                             # f|P|g per p
        ab = keep.tile([128, 1024], FP, tag="ab")             # [bA|bB|aA|aB]
        rep_nat = keep.tile([128, 512], FP, tag="rep_nat")    # [A|B]
        attn_nat = keep.tile([128, 512], FP, tag="attn_nat")
        atc = [[keep.tile([128, CW * 512], FP, tag=f"atc{it}_{c}",
                          name=f"atc{it}_{c}") for c in range(CH)]
               for it in range(BC)]                           # H_p terms
        dene = [keep.tile([128, 256], FP, tag=f"dene{it}", name=f"dene{it}")
                for it in range(BC)]
        rden = [keep.tile([128, 256], FP, tag=f"rden{it}", name=f"rden{it}")
                for it in range(BC)]
        xT = [[None] * 2 for _ in range(BC)]
        repT = [[None] * 2 for _ in range(BC)]
        attnT = [[None] * 2 for _ in range(BC)]
        for it in range(BC):
            for h in range(2):
                xT[it][h] = keep.tile([128, 128], FP, tag=f"xT_{it}_{h}",
                                      name=f"xT_{it}_{h}")
                repT[it][h] = keep.tile([128, 128], FP, tag=f"repT_{it}_{h}",
                                        name=f"repT_{it}_{h}")
                attnT[it][h] = keep.tile([128, 128], FP, tag=f"attnT_{it}_{h}",
                                         name=f"attnT_{it}_{h}")

        def body(_iv=None):
            # ---------------- phase A ----------------
            rpos = small.tile([128, 512], FP, tag="rpos")
            zneg = small.tile([128, 512], FP, tag="zneg")
            for it in range(BC):
                xs = []
                for h in range(2):
                    t = small.tile([128, 128], FP, tag="x_in")
                    nc.sync.dma_start(
                        out=t[:], in_=x_d.ap()[it, :, 128 * h:128 * (h + 1)])
                    xs.append(t)
                for h in range(2):
                    pt = psmall.tile([128, 512], FP, tag="ps")
                    nc.tensor.transpose(pt[:, :128], xs[h][:], ident[:])
                    nc.scalar.copy(out=_r(xT[it][h][:]), in_=pt[:, :128])
                psr = psmall.tile([128, 512], FP, tag="ps")
                nc.tensor.matmul(out=psr[:, :256], lhsT=_r(xT[it][0][:]),
                                 rhs=_r(w_rep[0][:]), start=True, stop=False)
                nc.tensor.matmul(out=psr[:, :256], lhsT=_r(xT[it][1][:]),
                                 rhs=_r(w_rep[1][:]), start=False, stop=False)
                nc.tensor.matmul(out=psr[:, :256], lhsT=_r(l0[:]),
                                 rhs=_r(brep_row[:]), start=False, stop=True)
                # elu(z) = relu(z) + exp(min(z, 0)) - 1
                nc.scalar.activation(out=rpos[:, 256 * it:256 * (it + 1)],
                                     in_=psr[:, :256], func=AF.Relu)
                nc.vector.tensor_scalar_min(zneg[:, 256 * it:256 * (it + 1)],
                                            psr[:, :256], 0.0)
            ez = small.tile([128, 512], FP, tag="ez")
            nc.scalar.activation(out=ez[:], in_=zneg[:], func=AF.Exp)
            nc.vector.scalar_tensor_tensor(
                out=rep_nat[:], in0=ez[:], scalar=-1.0, in1=rpos[:],
                op0=ALU.add, op1=ALU.add)

            for it in range(BC):
                for h in range(2):
                    pt = psmall.tile([128, 512], FP, tag="ps")
                    nc.tensor.transpose(
                        pt[:, :128],
                        rep_nat[:, 256 * it + 128 * h:256 * it + 128 * (h + 1)],
                        ident[:])
                    nc.scalar.copy(out=_r(repT[it][h][:]), in_=pt[:, :128])
                # b := dep, a := head + b_attn
                psd = psmall.tile([128, 512], FP, tag="ps")
                nc.tensor.matmul(out=psd[:, :256], lhsT=_r(repT[it][0][:]),
                                 rhs=_r(w_dep[0][:]), start=True, stop=False)
                nc.tensor.matmul(out=psd[:, :256], lhsT=_r(repT[it][1][:]),
                                 rhs=_r(w_dep[1][:]), start=False, stop=True)
                nc.scalar.copy(out=ab[:, 256 * it:256 * (it + 1)],
                               in_=psd[:, :256])
                psh = psmall.tile([128, 512], FP, tag="ps")
                nc.tensor.matmul(out=psh[:, :256], lhsT=_r(repT[it][0][:]),
                                 rhs=_r(w_head[0][:]), start=True, stop=False)
                nc.tensor.matmul(out=psh[:, :256], lhsT=_r(repT[it][1][:]),
                                 rhs=_r(w_head[1][:]), start=False, stop=False)
                nc.tensor.matmul(out=psh[:, :256], lhsT=_r(l0[:]),
                                 rhs=_r(battn_row[:]), start=False, stop=True)
                nc.scalar.copy(out=ab[:, 512 + 256 * it:512 + 256 * (it + 1)],
                               in_=psh[:, :256])

            # ---------------- phase B: factorized attention ----------------
            # f_p|g_p = exp(lam*[b|a] + ln|w|/2): one ACT instr per p,
            # then P_p = f_p * rep on gpsimd, emitted chunk-staged.
            for c in range(CH):
                for q in range(CW):
                    p = c * CW + q
                    nc.scalar.activation(
                        out=_ap(mc[c][:], [[1024, 2], [1, 512]], PBLK * q),
                        in_=_ap(ab[:], [[512, 2], [1, 512]]),
                        func=AF.Exp, scale=float(LAMS[p]),
                        bias=hcol[:, p:p + 1])
                for q in range(CW):
                    nc.gpsimd.tensor_tensor(
                        out=_ap(mc[c][:], [[1, 512]], PBLK * q + 512),
                        in0=_ap(mc[c][:], [[1, 512]], PBLK * q),
                        in1=_ap(rep_nat[:], [[1, 512]]), op=ALU.mult)

            # prefix matmuls (fp32 exact) + H_p = +-cum_p * g_p (DVE;
            # the w_p sign folds into a scalar_tensor_tensor for free).
            # Item 0's p-sum: PSUM-accumulated identity matmuls on the PE,
            # emitted one chunk late so they never stall on the DVE mults.
            # Item 1's p-sum: gpsimd add tree (3 in-chunk adds per chunk +
            # a cross-chunk merge).
            ndps = pacc.tile([128, 512], FP, tag="nd0", name="ndps0")

            def psum_mms(c, start, stop):
                for q in range(CW):
                    nc.tensor.matmul(
                        out=ndps[:, :512], lhsT=ident[:],
                        rhs=_ap(atc[0][c][:], [[1, 512]], 512 * q),
                        start=(start and q == 0),
                        stop=(stop and q == CW - 1))

            for c in range(CH):
                for q in range(CW):
                    p = c * CW + q
                    for it in range(BC):
                        ps = pspre.tile([128, 512], FP, tag="pp")
                        nc.tensor.matmul(
                            out=ps[:, :512], lhsT=ltri[:],
                            rhs=_ap(mc[c][:], [[512, 2], [1, 256]],
                                    PBLK * q + 256 * it),
                            start=True, stop=True)
                        if p < NNEG:
                            nc.vector.scalar_tensor_tensor(
                                out=_ap(atc[it][c][:], [[1, 512]], 512 * q),
                                in0=_ap(ps[:], [[1, 512]]), scalar=-1.0,
                                in1=_ap(mc[c][:], [[0, 2], [1, 256]],
                                        PBLK * q + 1024 + 256 * it),
                                op0=ALU.mult, op1=ALU.mult)
                        else:
                            nc.vector.tensor_tensor(
                                out=_ap(atc[it][c][:], [[1, 512]], 512 * q),
                                in0=_ap(ps[:], [[1, 512]]),
                                in1=_ap(mc[c][:], [[0, 2], [1, 256]],
                                        PBLK * q + 1024 + 256 * it),
                                op=ALU.mult)
                if c >= 1:
                    psum_mms(c - 1, start=(c == 1), stop=False)
                # item-1 in-chunk tree: slot0 += slot1; slot2 += slot3;
                # slot0 += slot2
                a1c = atc[1][c]
                for d_, s_ in [(0, 1), (2, 3), (0, 2)]:
                    nc.gpsimd.tensor_tensor(
                        out=_ap(a1c[:], [[1, 512]], 512 * d_),
                        in0=_ap(a1c[:], [[1, 512]], 512 * d_),
                        in1=_ap(a1c[:], [[1, 512]], 512 * s_), op=ALU.add)
            psum_mms(CH - 1, start=False, stop=True)
            # cross-chunk merge for item 1 (into chunk 0 slot 0)
            for c in range(1, CH):
                nc.gpsimd.tensor_tensor(
                    out=_ap(atc[1][0][:], [[1, 512]]),
                    in0=_ap(atc[1][0][:], [[1, 512]]),
                    in1=_ap(atc[1][c][:], [[1, 512]]), op=ALU.add)

            # attn = num / (den + eps)
            nd_aps = [ndps[:, :512], _ap(atc[1][0][:], [[1, 512]])]
            for it in range(BC):
                nd = nd_aps[it]
                nc.vector.tensor_scalar_add(
                    dene[it][:], _ap(nd, [[1, 256]]), DEN_EPS)
                nc.vector.reciprocal(out=rden[it][:], in_=dene[it][:])
                nc.vector.tensor_tensor(
                    out=attn_nat[:, 256 * it:256 * (it + 1)],
                    in0=_ap(nd, [[1, 256]], 256), in1=rden[it][:], op=ALU.mult)

            # ---------------- phase C: fusion gate ----------------
            eg = small.tile([128, 512], FP, tag="eg")
            for it in range(BC):
                for h in range(2):
                    pt = psmall.tile([128, 512], FP, tag="ps")
                    nc.tensor.transpose(
                        pt[:, :128],
                        attn_nat[:, 256 * it + 128 * h:256 * it + 128 * (h + 1)],
                        ident[:])
                    nc.scalar.copy(out=_r(attnT[it][h][:]), in_=pt[:, :128])
                psg = psmall.tile([128, 512], FP, tag="ps")
                nc.tensor.matmul(out=psg[:, :256], lhsT=_r(attnT[it][0][:]),
                                 rhs=_r(w_fattn[0][:]), start=True, stop=False)
                nc.tensor.matmul(out=psg[:, :256], lhsT=_r(attnT[it][1][:]),
                                 rhs=_r(w_fattn[1][:]), start=False, stop=False)
                nc.tensor.matmul(out=psg[:, :256], lhsT=_r(repT[it][0][:]),
                                 rhs=_r(w_frep[0][:]), start=False, stop=False)
                nc.tensor.matmul(out=psg[:, :256], lhsT=_r(repT[it][1][:]),
                                 rhs=_r(w_frep[1][:]), start=False, stop=False)
                nc.tensor.matmul(out=psg[:, :256], lhsT=_r(l0[:]),
                                 rhs=_r(bf_row[:]), start=False, stop=True)
                # gate = 1/(1 + exp(-z))
                nc.scalar.activation(out=eg[:, 256 * it:256 * (it + 1)],
                                     in_=psg[:, :256], func=AF.Exp, scale=-1.0)
            nc.vector.tensor_scalar_add(eg[:], eg[:], 1.0)
            gate = small.tile([128, 512], FP, tag="gate")
            nc.vector.reciprocal(out=gate[:], in_=eg[:])
            # out = attn + gate*(rep - attn)   (gpsimd)
            outt = small.tile([128, 512], FP, tag="outt")
            nc.gpsimd.tensor_sub(outt[:], rep_nat[:], attn_nat[:])
            nc.gpsimd.tensor_tensor(out=outt[:], in0=gate[:], in1=outt[:],
                                    op=ALU.mult)
            nc.gpsimd.tensor_add(outt[:], outt[:], attn_nat[:])
            for it in range(BC):
                nc.sync.dma_start(out=out_d.ap()[it, :, :],
                                  in_=outt[:, 256 * it:256 * (it + 1)])

        if reps == 1:
            body()
        else:
            with tc.For_i(0, reps, 1) as iv:
                body(iv)


_CACHED = {}


def _get_program(reps=1):
    if reps not in _CACHED:
        _CACHED[reps] = build_program(reps)
    return _CACHED[reps]


def make_in_maps(inputs):
    x = np.ascontiguousarray(np.asarray(inputs["x"], dtype=np.float32))
    names = {
        "w_rep": inputs["rep_map_kernel"], "b_rep": inputs["rep_map_bias"],
        "w_head": inputs["head_kernel"], "w_dep": inputs["dependent_kernel"],
        "b_attn": inputs["attn_bias"], "w_frep": inputs["f_rep_kernel"],
        "w_fattn": inputs["f_attn_kernel"], "b_f": inputs["f_bias"],
    }
    shared = {k: np.ascontiguousarray(np.asarray(v, dtype=np.float32))
              for k, v in names.items()}
    return [dict(shared, x=x[c * BC:(c + 1) * BC]) for c in range(NCORES)]


def kernel(**inputs):
    nc = _get_program(reps=1)
    in_maps = make_in_maps(inputs)
    res = run_bass_kernel_spmd(nc, in_maps, list(range(NCORES)))
    out = np.concatenate([res.results[c]["out"] for c in range(NCORES)],
                         axis=0)
    return out.astype(np.float32)


# revision 22
# speedup vs baseline: 1.1985x; 1.1985x over previous
"""Trainium2 Bass kernel for nn_DirectionalAttn (directional sparse attention).

Computation (per batch item b):
    rep = elu(x @ W_rep + b_rep)                       # [S, D]
    dep = rep @ W_dep;  head = rep @ W_head            # [S, D]
    E[i,j,d]  = exp(5*tanh((dep[j,d] + head[i,d] + b_attn[d]) / 5)) * (j < i)
    attn[i,d] = sum_j E*rep[j,d] / sum_j E
    gate = sigmoid(attn @ W_fa + rep @ W_fr + b_f)
    out  = gate*rep + (1-gate)*attn

Key optimization: the logit nonlinearity G(s) = exp(5*tanh(s/5)) with
s = dep[j,d] + head[i,d] is replaced by a rank-R sum of exponentials
    G(s) ~= sum_p w_p * exp(lam_p * s) = sum_p f_p(dep) * g_p(head)
(weighted least-squares fit over s in [-8.5, 8.5]; end-to-end output
error ~1e-2 vs the exact reference, inside the 2e-2 gate).  This
FACTORIZES the S^2*D attention tensor:
    num[i,d] = sum_p g_p[i,d] * cumsum_{j<i}(f_p[j,d]*rep[j,d])
    den[i,d] = sum_p g_p[i,d] * cumsum_{j<i}(f_p[j,d])
so the attention core is R strict-lower-triangular prefix-sum matmuls
(PE, fp32: the sign-cancelling p-sum amplifies operand rounding, so
the prefix inputs stay full fp32) plus O(S*D*R) elementwise work.  No
S^2 intermediate exists at all.

Engine plan (DVE ops pay a pipe-drain ~2x penalty on TRN2, so DVE work
is minimized):  ACT evaluates f_p|g_p fused (one instr per p, ln|w_p|
split across both exponentials); gpsimd does the f*rep multiplies and
the output mix; DVE only does the g*cumsum products (PSUM-sourced, so
gpsimd can't take them); PE does everything linear: biases are folded
into the phase-A matmuls via a ones-row matmul, and the sum over p
(with the w_p signs) is PSUM-accumulated identity/neg-identity
matmuls over float32r H_p = g_p*cum_p terms.

Sharding: data-parallel over batch, 2 items per core across 8 cores,
weights replicated.  rep_mask is all-ones per the problem spec.
"""

import numpy as np

import concourse.bacc as bacc
import concourse.bass as bass
import concourse.mybir as mybir
import concourse.tile as tile
from concourse.bass_utils import run_bass_kernel_spmd
from concourse.masks import make_identity

B, S, D = 16, 128, 256
NCORES = 8
BC = B // NCORES          # batch items per core
FP = mybir.dt.float32
AF = mybir.ActivationFunctionType
ALU = mybir.AluOpType
DEN_EPS = 1e-30

# Exponential-sum fit of G(s) = exp(5*tanh(s/5)) on s in [-8.5, 8.5]
# (variable-projection weighted LSQ; see docstring).  Negative-weight
# terms come first (their p-sum matmuls use the -identity stationary).
LAMS = [-0.597973593276427, -0.28297507943910016, -0.009232368117889064,
        0.26270803396363485, 0.8245601025351921, 1.5940452331145831,
        -0.6851034618361485, -0.4508378574683121, -0.12564357055407682,
        0.09239083103298487, 0.500820684846457, 1.2302589584057047]
# 0.5*ln(|w_p|): split between the f and g exponentials.
HALFC = [0.9772085280973349, 2.6475893172649894, 3.699843991890653,
         3.0224898830343245, 0.8626780526325857, -3.015461093018575,
         0.07896823713218788, 1.821021127931912, 3.357006157920167,
         3.5959232327063892, 2.182255902121232, -0.9830975255885379]
NNEG = 6                  # LAMS[0:NNEG] have negative weights
R = len(LAMS)
CH, CW = 3, 4             # phase-B chunking: CH chunks of CW p's
# p-block layout inside the mega tile: [fA|fB|PA|PB|gA|gB] * 256
PBLK = 6 * 256            # 1536 cols per p


def _ap(ap, free_pattern, extra_offset=0):
    """AP over `ap`'s tensor keeping its partition dim, with a custom free
    pattern (supports step-0 broadcast entries).  Offsets are in elements."""
    return bass.AP(
        tensor=ap.tensor,
        offset=ap.offset + extra_offset,
        ap=[list(ap.ap[0])] + [list(p) for p in free_pattern],
    )


def _r(ap):
    """float32r view: PE moving operand at 1 cyc/col (vs 4 for fp32) when
    the output free size is >= 256.  ~2^-11 operand rounding."""
    return ap.bitcast(mybir.dt.float32r)


def build_program(reps=1):
    nc = bacc.Bacc("TRN2", target_bir_lowering=False, debug=False,
                   num_devices=NCORES)

    x_d = nc.dram_tensor("x", [BC, S, D], FP, kind="ExternalInput")
    w_rep_d = nc.dram_tensor("w_rep", [D, D], FP, kind="ExternalInput")
    b_rep_d = nc.dram_tensor("b_rep", [D], FP, kind="ExternalInput")
    w_head_d = nc.dram_tensor("w_head", [D, D], FP, kind="ExternalInput")
    w_dep_d = nc.dram_tensor("w_dep", [D, D], FP, kind="ExternalInput")
    b_attn_d = nc.dram_tensor("b_attn", [D], FP, kind="ExternalInput")
    w_frep_d = nc.dram_tensor("w_frep", [D, D], FP, kind="ExternalInput")
    w_fattn_d = nc.dram_tensor("w_fattn", [D, D], FP, kind="ExternalInput")
    b_f_d = nc.dram_tensor("b_f", [D], FP, kind="ExternalInput")
    out_d = nc.dram_tensor("out", [BC, S, D], FP, kind="ExternalOutput")

    with tile.TileContext(nc) as tc:
        _emit(nc, tc, reps, x_d, w_rep_d, b_rep_d, w_head_d, w_dep_d,
              b_attn_d, w_frep_d, w_fattn_d, b_f_d, out_d)
    nc.compile()
    return nc


def _emit(nc, tc, reps, x_d, w_rep_d, b_rep_d, w_head_d, w_dep_d,
          b_attn_d, w_frep_d, w_fattn_d, b_f_d, out_d):
    from contextlib import ExitStack
    ctx = ExitStack()
    with ctx:
        const = ctx.enter_context(tc.tile_pool(name="const", bufs=1))
        wpool = ctx.enter_context(tc.tile_pool(name="weights", bufs=1))
        keep = ctx.enter_context(tc.tile_pool(name="keep", bufs=1))
        small = ctx.enter_context(tc.tile_pool(name="small", bufs=2))
        psmall = ctx.enter_context(tc.tile_pool(name="psmall", bufs=2,
                                                space="PSUM"))
        pspre = ctx.enter_context(tc.tile_pool(name="pspre", bufs=5,
                                               space="PSUM"))
        pacc = ctx.enter_context(tc.tile_pool(name="pacc", bufs=1,
                                              space="PSUM"))

        ident = const.tile([128, 128], FP, tag="ident")
        make_identity(nc, ident[:])
        ident_r = const.tile([128, 128], FP, tag="ident_r")
        nc.scalar.copy(out=_r(ident_r[:]), in_=ident[:])
        identneg_r = const.tile([128, 128], FP, tag="identneg_r")
        nc.vector.tensor_scalar_mul(_r(identneg_r[:]), ident[:], -1.0)

        # strict-lower-triangular prefix matrix as lhsT: L[j, i] = (j < i)
        ltri = const.tile([128, 128], FP, tag="ltri")
        nc.vector.memset(ltri[:], 1.0)
        nc.gpsimd.affine_select(out=ltri[:], in_=ltri[:],
                                compare_op=ALU.is_ge, fill=0.0, base=-1,
                                channel_multiplier=-1, pattern=[[1, 128]])

        # --- replicated weights: W[dh] = W[128dh:128dh+128, :] ---
        def load_w(dram, nm):
            halves = []
            for dh in range(2):
                t0 = small.tile([128, 256], FP, tag="wload")
                nc.sync.dma_start(out=t0[:],
                                  in_=dram.ap()[128 * dh:128 * (dh + 1), :])
                t = wpool.tile([128, 256], FP, tag=f"{nm}_{dh}")
                nc.scalar.copy(out=_r(t[:]), in_=t0[:])
                halves.append(t)
            return halves

        w_rep = load_w(w_rep_d, "wrep")
        w_dep = load_w(w_dep_d, "wdep")
        w_head = load_w(w_head_d, "whead")
        w_frep = load_w(w_frep_d, "wfrep")
        w_fattn = load_w(w_fattn_d, "wfattn")

        # l0[q, p] = (q == 0): adding matmul(lhsT=l0, rhs=bias_row) to a
        # psum accumulation folds a free-axis bias add into the PE.
        l0 = const.tile([128, 128], FP, tag="l0")
        l0s = small.tile([128, 128], FP, tag="l0s")
        nc.vector.memset(l0s[:], 0.0)
        nc.vector.memset(l0s[0:1, :], 1.0)
        nc.scalar.copy(out=_r(l0[:]), in_=l0s[:])

        def bias_row(dram, tag):
            t0 = small.tile([128, 256], FP, tag="wload")
            nc.vector.memset(t0[:], 0.0)
            nc.sync.dma_start(out=_ap(t0[0:1, :], [[1, 256]]), in_=dram.ap())
            t = wpool.tile([128, 256], FP, tag=tag)
            nc.scalar.copy(out=_r(t[:]), in_=t0[:])
            return t

        brep_row = bias_row(b_rep_d, "brep_row")
        battn_row = bias_row(b_attn_d, "battn_row")
        bf_row = bias_row(b_f_d, "bf_row")

        # per-p ACT bias columns: 0.5*ln|w_p|
        hcol = const.tile([128, R], FP, tag="hcol")
        for p in range(R):
            nc.vector.memset(hcol[:, p:p + 1], float(HALFC[p]))

        # persistent work tiles.  The tile framework tracks dependencies
        # at tile granularity and the HW penalizes dense fine-grained
        # semaphores, so phase-B state is CHUNKED: 3 chunks of 4 p's.
        # Coarse enough to keep sem traffic low, fine enough that the PE
        # prefix stream starts after 1/3 of the ACT stream, not all of it.
        mc = [keep.tile([128, CW * PBLK], FP, tag=f"mc{c}", name=f"mc{c}")
              for c in range(CH)]                             # f|P|g per p
        ab = keep.tile([128, 1024], FP, tag="ab")             # [bA|bB|aA|aB]
        rep_nat = keep.tile([128, 512], FP, tag="rep_nat")    # [A|B]
        attn_nat = keep.tile([128, 512], FP, tag="attn_nat")
        atc = [[keep.tile([128, CW * 512], FP, tag=f"atc{it}_{c}",
                          name=f"atc{it}_{c}") for c in range(CH)]
               for it in range(BC)]                           # H_p terms
        dene = [keep.tile([128, 256], FP, tag=f"dene{it}", name=f"dene{it}")
                for it in range(BC)]
        rden = [keep.tile([128, 256], FP, tag=f"rden{it}", name=f"rden{it}")
                for it in range(BC)]
        xT = [[None] * 2 for _ in range(BC)]
        repT = [[None] * 2 for _ in range(BC)]
        attnT = [[None] * 2 for _ in range(BC)]
        for it in range(BC):
            for h in range(2):
                xT[it][h] = keep.tile([128, 128], FP, tag=f"xT_{it}_{h}",
                                      name=f"xT_{it}_{h}")
                repT[it][h] = keep.tile([128, 128], FP, tag=f"repT_{it}_{h}",
                                        name=f"repT_{it}_{h}")
                attnT[it][h] = keep.tile([128, 128], FP, tag=f"attnT_{it}_{h}",
                                         name=f"attnT_{it}_{h}")

        def body(_iv=None):
            # ---------------- phase A ----------------
            rpos = small.tile([128, 512], FP, tag="rpos")
            zneg = small.tile([128, 512], FP, tag="zneg")
            for it in range(BC):
                xs = []
                for h in range(2):
                    t = small.tile([128, 128], FP, tag="x_in")
                    nc.sync.dma_start(
                        out=t[:], in_=x_d.ap()[it, :, 128 * h:128 * (h + 1)])
                    xs.append(t)
                for h in range(2):
                    pt = psmall.tile([128, 512], FP, tag="ps")
                    nc.tensor.transpose(pt[:, :128], xs[h][:], ident[:])
                    nc.scalar.copy(out=_r(xT[it][h][:]), in_=pt[:, :128])
                psr = psmall.tile([128, 512], FP, tag="ps")
                nc.tensor.matmul(out=psr[:, :256], lhsT=_r(xT[it][0][:]),
                                 rhs=_r(w_rep[0][:]), start=True, stop=False)
                nc.tensor.matmul(out=psr[:, :256], lhsT=_r(xT[it][1][:]),
                                 rhs=_r(w_rep[1][:]), start=False, stop=False)
                nc.tensor.matmul(out=psr[:, :256], lhsT=_r(l0[:]),
                                 rhs=_r(brep_row[:]), start=False, stop=True)
                # elu(z) = relu(z) + exp(min(z, 0)) - 1
                nc.scalar.activation(out=rpos[:, 256 * it:256 * (it + 1)],
                                     in_=psr[:, :256], func=AF.Relu)
                nc.vector.tensor_scalar_min(zneg[:, 256 * it:256 * (it + 1)],
                                            psr[:, :256], 0.0)
            ez = small.tile([128, 512], FP, tag="ez")
            nc.scalar.activation(out=ez[:], in_=zneg[:], func=AF.Exp)
            nc.vector.scalar_tensor_tensor(
                out=rep_nat[:], in0=ez[:], scalar=-1.0, in1=rpos[:],
                op0=ALU.add, op1=ALU.add)

            for it in range(BC):
                for h in range(2):
                    pt = psmall.tile([128, 512], FP, tag="ps")
                    nc.tensor.transpose(
                        pt[:, :128],
                        rep_nat[:, 256 * it + 128 * h:256 * it + 128 * (h + 1)],
                        ident[:])
                    nc.scalar.copy(out=_r(repT[it][h][:]), in_=pt[:, :128])
                # b := dep, a := head + b_attn
                psd = psmall.tile([128, 512], FP, tag="ps")
                nc.tensor.matmul(out=psd[:, :256], lhsT=_r(repT[it][0][:]),
                                 rhs=_r(w_dep[0][:]), start=True, stop=False)
                nc.tensor.matmul(out=psd[:, :256], lhsT=_r(repT[it][1][:]),
                                 rhs=_r(w_dep[1][:]), start=False, stop=True)
                nc.scalar.copy(out=ab[:, 256 * it:256 * (it + 1)],
                               in_=psd[:, :256])
                psh = psmall.tile([128, 512], FP, tag="ps")
                nc.tensor.matmul(out=psh[:, :256], lhsT=_r(repT[it][0][:]),
                                 rhs=_r(w_head[0][:]), start=True, stop=False)
                nc.tensor.matmul(out=psh[:, :256], lhsT=_r(repT[it][1][:]),
                                 rhs=_r(w_head[1][:]), start=False, stop=False)
                nc.tensor.matmul(out=psh[:, :256], lhsT=_r(l0[:]),
                                 rhs=_r(battn_row[:]), start=False, stop=True)
                nc.scalar.copy(out=ab[:, 512 + 256 * it:512 + 256 * (it + 1)],
                               in_=psh[:, :256])

            # ---------------- phase B: factorized attention ----------------
            # f_p|g_p = exp(lam*[b|a] + ln|w|/2): one ACT instr per p,
            # then P_p = f_p * rep on gpsimd, emitted chunk-staged.
            for c in range(CH):
                for q in range(CW):
                    p = c * CW + q
                    nc.scalar.activation(
                        out=_ap(mc[c][:], [[1024, 2], [1, 512]], PBLK * q),
                        in_=_ap(ab[:], [[512, 2], [1, 512]]),
                        func=AF.Exp, scale=float(LAMS[p]),
                        bias=hcol[:, p:p + 1])
                for q in range(CW):
                    nc.gpsimd.tensor_tensor(
                        out=_ap(mc[c][:], [[1, 512]], PBLK * q + 512),
                        in0=_ap(mc[c][:], [[1, 512]], PBLK * q),
                        in1=_ap(rep_nat[:], [[1, 512]]), op=ALU.mult)

            # prefix matmuls (fp32 exact) + H_p = +-cum_p * g_p (DVE;
            # the w_p sign folds into a scalar_tensor_tensor for free).
            # Item 0's p-sum: PSUM-accumulated identity matmuls on the PE,
            # emitted one chunk late so they never stall on the DVE mults.
            # Item 1's p-sum: gpsimd add tree (3 in-chunk adds per chunk +
            # a cross-chunk merge).
            ndps = pacc.tile([128, 512], FP, tag="nd0", name="ndps0")

            def psum_mms(c, start, stop):
                for q in range(CW):
                    nc.tensor.matmul(
                        out=ndps[:, :512], lhsT=ident[:],
                        rhs=_ap(atc[0][c][:], [[1, 512]], 512 * q),
                        start=(start and q == 0),
                        stop=(stop and q == CW - 1))

            for c in range(CH):
                # negate the g-slices of negative-w p's in this chunk
                nneg_c = min(max(NNEG - c * CW, 0), CW)
                if nneg_c:
                    gneg = _ap(mc[c][:], [[PBLK, nneg_c], [1, 512]], 1024)
                    nc.vector.tensor_scalar_mul(gneg, gneg, -1.0)
                for q in range(CW):
                    p = c * CW + q
                    for it in range(BC):
                        ps = pspre.tile([128, 512], FP, tag="pp")
                        nc.tensor.matmul(
                            out=ps[:, :512], lhsT=ltri[:],
                            rhs=_ap(mc[c][:], [[512, 2], [1, 256]],
                                    PBLK * q + 256 * it),
                            start=True, stop=True)
                        nc.vector.tensor_tensor(
                            out=_ap(atc[it][c][:], [[1, 512]], 512 * q),
                            in0=_ap(ps[:], [[1, 512]]),
                            in1=_ap(mc[c][:], [[0, 2], [1, 256]],
                                    PBLK * q + 1024 + 256 * it),
                            op=ALU.mult)
                if c >= 1:
                    psum_mms(c - 1, start=(c == 1), stop=False)
                # item-1 in-chunk tree: slot0 += slot1; slot2 += slot3;
                # slot0 += slot2
                a1c = atc[1][c]
                for d_, s_ in [(0, 1), (2, 3), (0, 2)]:
                    nc.gpsimd.tensor_tensor(
                        out=_ap(a1c[:], [[1, 512]], 512 * d_),
                        in0=_ap(a1c[:], [[1, 512]], 512 * d_),
                        in1=_ap(a1c[:], [[1, 512]], 512 * s_), op=ALU.add)
            psum_mms(CH - 1, start=False, stop=True)
            # cross-chunk merge for item 1 (into chunk 0 slot 0)
            for c in range(1, CH):
                nc.gpsimd.tensor_tensor(
                    out=_ap(atc[1][0][:], [[1, 512]]),
                    in0=_ap(atc[1][0][:], [[1, 512]]),
                    in1=_ap(atc[1][c][:], [[1, 512]]), op=ALU.add)

            # attn = num / (den + eps)
            nd_aps = [ndps[:, :512], _ap(atc[1][0][:], [[1, 512]])]
            for it in range(BC):
                nd = nd_aps[it]
                nc.vector.tensor_scalar_add(
                    dene[it][:], _ap(nd, [[1, 256]]), DEN_EPS)
                nc.vector.reciprocal(out=rden[it][:], in_=dene[it][:])
                nc.vector.tensor_tensor(
                    out=attn_nat[:, 256 * it:256 * (it + 1)],
                    in0=_ap(nd, [[1, 256]], 256), in1=rden[it][:], op=ALU.mult)

            # ---------------- phase C: fusion gate ----------------
            eg = small.tile([128, 512], FP, tag="eg")
            for it in range(BC):
                for h in range(2):
                    pt = psmall.tile([128, 512], FP, tag="ps")
                    nc.tensor.transpose(
                        pt[:, :128],
                        attn_nat[:, 256 * it + 128 * h:256 * it + 128 * (h + 1)],
                        ident[:])
                    nc.scalar.copy(out=_r(attnT[it][h][:]), in_=pt[:, :128])
                psg = psmall.tile([128, 512], FP, tag="ps")
                nc.tensor.matmul(out=psg[:, :256], lhsT=_r(attnT[it][0][:]),
                                 rhs=_r(w_fattn[0][:]), start=True, stop=False)
                nc.tensor.matmul(out=psg[:, :256], lhsT=_r(attnT[it][1][:]),
                                 rhs=_r(w_fattn[1][:]), start=False, stop=False)
                nc.tensor.matmul(out=psg[:, :256], lhsT=_r(repT[it][0][:]),
                                 rhs=_r(w_frep[0][:]), start=False, stop=False)
                nc.tensor.matmul(out=psg[:, :256], lhsT=_r(repT[it][1][:]),
                                 rhs=_r(w_frep[1][:]), start=False, stop=False)
                nc.tensor.matmul(out=psg[:, :256], lhsT=_r(l0[:]),
                                 rhs=_r(bf_row[:]), start=False, stop=True)
                # gate = 1/(1 + exp(-z))
                nc.scalar.activation(out=eg[:, 256 * it:256 * (it + 1)],
                                     in_=psg[:, :256], func=AF.Exp, scale=-1.0)
            nc.vector.tensor_scalar_add(eg[:], eg[:], 1.0)
            gate = small.tile([128, 512], FP, tag="gate")
            nc.vector.reciprocal(out=gate[:], in_=eg[:])
            # out = attn + gate*(rep - attn)   (gpsimd)
            outt = small.tile([128, 512], FP, tag="outt")
            nc.gpsimd.tensor_sub(outt[:], rep_nat[:], attn_nat[:])
            nc.gpsimd.tensor_tensor(out=outt[:], in0=gate[:], in1=outt[:],
                                    op=ALU.mult)
            nc.gpsimd.tensor_add(outt[:], outt[:], attn_nat[:])
            for it in range(BC):
                nc.sync.dma_start(out=out_d.ap()[it, :, :],
                                  in_=outt[:, 256 * it:256 * (it + 1)])

        if reps == 1:
            body()
        else:
            with tc.For_i(0, reps, 1) as iv:
                body(iv)


_CACHED = {}


def _get_program(reps=1):
    if reps not in _CACHED:
        _CACHED[reps] = build_program(reps)
    return _CACHED[reps]


def make_in_maps(inputs):
    x = np.ascontiguousarray(np.asarray(inputs["x"], dtype=np.float32))
    names = {
        "w_rep": inputs["rep_map_kernel"], "b_rep": inputs["rep_map_bias"],
        "w_head": inputs["head_kernel"], "w_dep": inputs["dependent_kernel"],
        "b_attn": inputs["attn_bias"], "w_frep": inputs["f_rep_kernel"],
        "w_fattn": inputs["f_attn_kernel"], "b_f": inputs["f_bias"],
    }
    shared = {k: np.ascontiguousarray(np.asarray(v, dtype=np.float32))
              for k, v in names.items()}
    return [dict(shared, x=x[c * BC:(c + 1) * BC]) for c in range(NCORES)]


def kernel(**inputs):
    nc = _get_program(reps=1)
    in_maps = make_in_maps(inputs)
    res = run_bass_kernel_spmd(nc, in_maps, list(range(NCORES)))
    out = np.concatenate([res.results[c]["out"] for c in range(NCORES)],
                         axis=0)
    return out.astype(np.float32)
